# revision 2
# baseline (speedup 1.0000x reference)
"""Trainium2 Bass kernel: 2048-point Hadamard transform.

out = (value @ H2048) * 2^-5.5,  value: (32768, 2048) f32.
H2048[n, m] = (-1)^popcount(n & m) = H128[n_lo7, m_lo7] * H16[n_hi4, m_hi4].

Token-streaming design with a PE-transpose crossing. Per 512-token group
(per core, 8 groups of 512 tokens):

  load   V[p, gi, t]      2 MB HBM->SBUF, p = n%128, gi = n//128
                          (16 KB/partition contiguous -> max descriptors)
  MM-A   A_gi[q, t] = sum_p (H128[p,q]*2^-4) V[p, gi, t]    16x N=512,
         lhsT = W1 constant; PSUM f32, 2-bank tiles
  drainA A -> S[q, gi, t] bf16                  (1024-elem f32 drains)
  T      64 PE transposes of S[:, (gi,t8-chunk)] -> tp2[(gi,t8), q] bf16
         (the 4-bit crossing; 8 tokens per chunk ride the partition dim)
  drainT tp2 -> vt2[(gi,t8), c, q] bf16         (1024-elem 2x drains)
  MM-B   m2[(j,t8), (c4,q)] = sum_gi (H16[gi,j]*2^-1.5) vt2[.., c4, q]
         16x N=512, lhsT = W2 constant
  drainB m2 -> OUT[(j,t8), c, q] bf16; store per quarter-group (sync ring)

DRAM output layout [w=(j,t8), tg, c, q] maps to out[t, m]:
t = tg*512 + c*8 + t8, m = j*128 + q; the host applies the inverse
permutation during its bf16->f32 upcast. Engine budget per core: PE 256 MM
N=512 + 512 transposes ~93us, DVE/ACT ~70us each, DMA 32 MB HBM ~90us.
"""

import numpy as np
import ml_dtypes

import concourse.bass as bass
import concourse.mybir as mybir
import concourse.tile as tile
from concourse.bass_utils import run_bass_kernel_spmd

N_CORES = 8
T_FULL = 32768
N = 2048
T_CORE = T_FULL // N_CORES  # 4096
P = 128
ST = 512  # tokens per group
N_G = T_CORE // ST  # 8 groups

BF16 = mybir.dt.bfloat16
F32 = mybir.dt.float32


def _sylvester(n: int) -> np.ndarray:
    H = np.array([[1.0]], dtype=np.float64)
    while H.shape[0] < n:
        H = np.block([[H, H], [H, -H]])
    return H


def _host_consts() -> np.ndarray:
    """[128, 384] bf16: [W1 | W2 | I].

    W1[p, q] = H128[p, q] * 2^-4                      (exact in bf16)
    W2[gi*8+t8, j*8+t8'] = H16[gi, j] * 2^-1.5 * (t8==t8')
    """
    H128 = _sylvester(128)
    H16 = _sylvester(16)
    W1 = H128 * 2.0**-4
    W2 = np.zeros((128, 128))
    for gi in range(16):
        for t8 in range(8):
            for j in range(16):
                W2[gi * 8 + t8, j * 8 + t8] = H16[gi, j] * 2.0**-1.5
    ident = np.eye(128)
    return np.concatenate([W1, W2, ident], axis=1).astype(ml_dtypes.bfloat16)


def build_bass(t_core: int = T_CORE) -> bass.Bass:
    n_g = t_core // ST
    nc = bass.Bass()
    vt_p = nc.declare_dram_parameter("vt", [P, n_g, 16, ST], BF16, isOutput=False)
    consts_p = nc.declare_dram_parameter("consts", [P, 3 * P], BF16, isOutput=False)
    out_p = nc.declare_dram_parameter("out", [P, n_g, 64, P], BF16, isOutput=True)

    with tile.TileContext(nc) as tc:
        with (
            tc.tile_pool(name="consts", bufs=1) as consts,
            tc.tile_pool(name="vpool", bufs=2) as vpool,
            tc.tile_pool(name="spool", bufs=2) as spool,
            tc.tile_pool(name="v2pool", bufs=2) as v2pool,
            tc.tile_pool(name="opool", bufs=2) as opool,
            tc.tile_pool(name="apsum", bufs=2, space="PSUM") as apsum,
            tc.tile_pool(name="tpsum", bufs=2, space="PSUM") as tpsum,
            tc.tile_pool(name="mpsum", bufs=2, space="PSUM") as mpsum,
        ):
            CONSTS = consts.tile([P, 3 * P], BF16, tag="consts")
            nc.scalar.dma_start(out=CONSTS, in_=consts_p[:, :])
            W1 = CONSTS[:, 0:P]
            W2 = CONSTS[:, P : 2 * P]
            IDENT = CONSTS[:, 2 * P : 3 * P]

            VS = [None] * n_g
            SS = [None] * n_g

            def issue_load(tg):
                vtile = vpool.tile([P, 16, ST], BF16, tag="v")
                VS[tg] = vtile
                if tg == 0:
                    # split the first load into gi-pair chunks in consumption
                    # order so A-unit(0,0) starts after ~250KB, not 2MB
                    for gp in range(8):
                        nc.sync.dma_start(
                            out=vtile[:, gp * 2 : gp * 2 + 2, :],
                            in_=vt_p[:, tg, gp * 2 : gp * 2 + 2, :],
                        )
                else:
                    nc.sync.dma_start(out=vtile, in_=vt_p[:, tg, :, :])

            def emit_a_unit(tg, gp):
                V = VS[tg]
                S = SS[tg]
                A = apsum.tile([P, 2, ST], F32, tag="a")
                for k in range(2):
                    gi = gp * 2 + k
                    nc.tensor.matmul(
                        A[:, k], W1, V[:, gi, :], start=True, stop=True
                    )
                # 1024-elem f32->bf16 drain; ACT-biased split (DVE is
                # reserved mostly for the 2x-rate bf16 tp2 drains)
                dst = S[:, :, gp * 2 : gp * 2 + 2, :]
                src = A.rearrange("p k (c t) -> p c k t", t=8)
                if gp % 4 == 0:
                    nc.vector.tensor_copy(out=dst, in_=src)
                else:
                    nc.scalar.activation(
                        out=dst,
                        in_=src,
                        func=mybir.ActivationFunctionType.Copy,
                    )
                if gp == 7:
                    VS[tg] = None

            OB = [None, None]  # OUT, vt2 of the group in stage B

            def emit_b_block(tg, B):
                S = SS[tg]
                # chunk c = tokens [8c, 8c+8); 8 blocks of 8 chunks
                if B == 0:
                    OUT = opool.tile([P, 64, P], BF16, tag="o")
                    vt2 = v2pool.tile([P, 64, P], BF16, tag="v2")
                    OB[0], OB[1] = OUT, vt2
                OUT, vt2 = OB
                tp2 = tpsum.tile([P, 8, P], BF16, tag="t2")
                for cs in range(8):
                    c = B * 8 + cs
                    nc.tensor.transpose(tp2[:, cs], S[:, c, :, :], IDENT)
                # 1024-elem bf16 drain: always DVE (2x rate on bf16 PSUM)
                nc.vector.tensor_copy(
                    out=vt2[:, B * 8 : B * 8 + 8, :], in_=tp2
                )
                for k in range(2):
                    Q = B * 2 + k
                    m2 = mpsum.tile([P, 4, P], F32, tag="m2")
                    nc.tensor.matmul(
                        m2,
                        W2,
                        vt2[:, 4 * Q : 4 * Q + 4, :],
                        start=True,
                        stop=True,
                    )
                    # 512-elem f32 drain, alternating engines
                    if Q % 2 == 0:
                        nc.scalar.activation(
                            out=OUT[:, Q * 4 : Q * 4 + 4, :],
                            in_=m2,
                            func=mybir.ActivationFunctionType.Copy,
                        )
                    else:
                        nc.vector.tensor_copy(
                            out=OUT[:, Q * 4 : Q * 4 + 4, :], in_=m2
                        )
                last = tg == n_g - 1
                if last:
                    # taper the tail: store per block on the final group
                    nc.sync.dma_start(
                        out=out_p[:, tg, B * 8 : B * 8 + 8, :],
                        in_=OUT[:, B * 8 : B * 8 + 8, :],
                    )
                elif B % 2 == 1:
                    nc.sync.dma_start(
                        out=out_p[:, tg, (B - 1) * 8 : B * 8 + 8, :],
                        in_=OUT[:, (B - 1) * 8 : B * 8 + 8, :],
                    )
                if B == 7:
                    SS[tg] = None

            # fine-grained software pipeline: A-unit(tg, i) interleaved with
            # B-block(tg-1, i) so DVE/ACT see an even drain mix throughout
            issue_load(0)
            for tg in range(n_g):
                if tg + 1 < n_g:
                    issue_load(tg + 1)
                S = spool.tile([P, 64, 16, 8], BF16, tag="s")
                SS[tg] = S
                for i in range(8):
                    if tg > 0:
                        emit_b_block(tg - 1, i)
                    emit_a_unit(tg, i)
            for i in range(8):
                emit_b_block(n_g - 1, i)

    import bass_rust

    bass_rust.move_matmul_waits_to_ldweights(nc.m)
    bass_rust.generate_event_semaphores(nc)
    return nc


_CACHE = {}


def _make_in_maps(inputs) -> list:
    value = np.asarray(inputs["value"])
    assert value.shape == (T_FULL, N), value.shape
    vb = value.astype(ml_dtypes.bfloat16)
    consts = _host_consts()
    in_maps = []
    for c in range(N_CORES):
        vc = vb[c * T_CORE : (c + 1) * T_CORE]
        # [p, tg, gi, tt] = V[tg*512+tt, gi*128+p]
        vt4 = np.ascontiguousarray(
            vc.reshape(N_G, ST, 16, P).transpose(3, 0, 2, 1)
        )
        in_maps.append({"vt": vt4, "consts": consts})
    return in_maps


def _decode_out(res_list) -> np.ndarray:
    """[w=(j,t8), tg, c, q] per core -> full (T_FULL, N) f32."""
    outs = []
    for r in res_list:
        o = np.asarray(r["out"])  # [128, 8, 64, 128] bf16
        o5 = o.reshape(16, 8, N_G, 64, P)  # [j, t8, tg, c, q]
        o5 = o5.transpose(2, 3, 1, 0, 4)  # [tg, c, t8, j, q]
        outs.append(
            np.ascontiguousarray(o5, dtype=np.float32).reshape(T_CORE, N)
        )
    return np.concatenate(outs, axis=0)


def _probe_ok(out: np.ndarray, inputs, n_rows: int = 3) -> bool:
    value = np.asarray(inputs["value"])
    weight = np.asarray(inputs["weight"], dtype=np.float32)
    rows = np.linspace(0, T_FULL - 1, n_rows).astype(int)
    scale = np.float32(1.0 / np.sqrt(np.float32(weight.shape[0])))
    want = (np.asarray(value[rows], dtype=np.float32) @ weight) * scale
    got = out[rows]
    denom = max(float(np.abs(want).max()), 1e-30)
    rel = float(np.abs(got - want).max()) / denom
    return rel < 1.5e-2


def kernel(**inputs) -> np.ndarray:
    if "nc" not in _CACHE:
        _CACHE["nc"] = build_bass(T_CORE)
    nc = _CACHE["nc"]

    in_maps = _make_in_maps(inputs)
    try:
        out = None
        for attempt in range(2):
            res = run_bass_kernel_spmd(nc, in_maps, list(range(N_CORES)))
            o = _decode_out(res.results)
            if _probe_ok(o, inputs):
                out = o
                break
            print("kernel: probe mismatch on attempt", attempt, flush=True)
        if out is None:
            raise RuntimeError("bass kernel failed host probe twice")
        return out
    except Exception:
        import traceback

        traceback.print_exc()
        print("kernel: falling back to jax path", flush=True)
        import jax
        import jax.numpy as jnp

        value = np.asarray(inputs["value"], dtype=np.float32)
        devs = jax.devices()[:N_CORES]
        scale = np.float32(1.0 / np.sqrt(np.float32(N)))
        w = np.asarray(inputs["weight"], dtype=np.float32)
        outs = []
        for c in range(N_CORES):
            d = devs[c % len(devs)]
            f = jax.jit(lambda a, b: jnp.dot(a, b) * scale, device=d)
            outs.append(f(value[c * T_CORE : (c + 1) * T_CORE], w))
        return np.concatenate([np.asarray(o) for o in outs], axis=0).astype(
            np.float32
        )


# revision 3
# speedup vs baseline: 1.0111x; 1.0111x over previous
"""Trainium2 Bass kernel: 2048-point Hadamard transform.

out = (value @ H2048) * 2^-5.5,  value: (32768, 2048) f32.
H2048[n, m] = (-1)^popcount(n & m) = H128[n_lo7, m_lo7] * H16[n_hi4, m_hi4].

Token-streaming design with a PE-transpose crossing. Per 512-token group
(per core, 8 groups of 512 tokens):

  load   V[p, gi, t]      2 MB HBM->SBUF, p = n%128, gi = n//128
                          (16 KB/partition contiguous -> max descriptors)
  MM-A   A_gi[q, t] = sum_p (H128[p,q]*2^-4) V[p, gi, t]    16x N=512,
         lhsT = W1 constant; PSUM f32, 2-bank tiles
  drainA A -> S[q, gi, t] bf16                  (1024-elem f32 drains)
  T      64 PE transposes of S[:, (gi,t8-chunk)] -> tp2[(gi,t8), q] bf16
         (the 4-bit crossing; 8 tokens per chunk ride the partition dim)
  drainT tp2 -> vt2[(gi,t8), c, q] bf16         (1024-elem 2x drains)
  MM-B   m2[(j,t8), (c4,q)] = sum_gi (H16[gi,j]*2^-1.5) vt2[.., c4, q]
         16x N=512, lhsT = W2 constant
  drainB m2 -> OUT[(j,t8), c, q] bf16; store per quarter-group (sync ring)

DRAM output layout [w=(j,t8), tg, c, q] maps to out[t, m]:
t = tg*512 + c*8 + t8, m = j*128 + q; the host applies the inverse
permutation during its bf16->f32 upcast. Engine budget per core: PE 256 MM
N=512 + 512 transposes ~93us, DVE/ACT ~70us each, DMA 32 MB HBM ~90us.
"""

import numpy as np
import ml_dtypes

import concourse.bass as bass
import concourse.mybir as mybir
import concourse.tile as tile
from concourse.bass_utils import run_bass_kernel_spmd

N_CORES = 8
T_FULL = 32768
N = 2048
T_CORE = T_FULL // N_CORES  # 4096
P = 128
ST = 512  # tokens per group
N_G = T_CORE // ST  # 8 groups

BF16 = mybir.dt.bfloat16
F32 = mybir.dt.float32


def _sylvester(n: int) -> np.ndarray:
    H = np.array([[1.0]], dtype=np.float64)
    while H.shape[0] < n:
        H = np.block([[H, H], [H, -H]])
    return H


def _host_consts() -> np.ndarray:
    """[128, 384] bf16: [W1 | W2 | I].

    W1[p, q] = H128[p, q] * 2^-4                      (exact in bf16)
    W2[gi*8+t8, j*8+t8'] = H16[gi, j] * 2^-1.5 * (t8==t8')
    """
    H128 = _sylvester(128)
    H16 = _sylvester(16)
    W1 = H128 * 2.0**-4
    W2 = np.zeros((128, 128))
    for gi in range(16):
        for t8 in range(8):
            for j in range(16):
                W2[gi * 8 + t8, j * 8 + t8] = H16[gi, j] * 2.0**-1.5
    ident = np.eye(128)
    return np.concatenate([W1, W2, ident], axis=1).astype(ml_dtypes.bfloat16)


def build_bass(t_core: int = T_CORE) -> bass.Bass:
    n_g = t_core // ST
    nc = bass.Bass()
    vt_p = nc.declare_dram_parameter("vt", [P, n_g, 16, ST], BF16, isOutput=False)
    consts_p = nc.declare_dram_parameter("consts", [P, 3 * P], BF16, isOutput=False)
    out_p = nc.declare_dram_parameter("out", [P, n_g, 64, P], BF16, isOutput=True)

    with tile.TileContext(nc) as tc:
        with (
            tc.tile_pool(name="consts", bufs=1) as consts,
            tc.tile_pool(name="vpool", bufs=2) as vpool,
            tc.tile_pool(name="spool", bufs=3) as spool,
            tc.tile_pool(name="v2pool", bufs=2) as v2pool,
            tc.tile_pool(name="opool", bufs=2) as opool,
            tc.tile_pool(name="apsum", bufs=2, space="PSUM") as apsum,
            tc.tile_pool(name="tpsum", bufs=2, space="PSUM") as tpsum,
            tc.tile_pool(name="mpsum", bufs=2, space="PSUM") as mpsum,
        ):
            CONSTS = consts.tile([P, 3 * P], BF16, tag="consts")
            nc.scalar.dma_start(out=CONSTS, in_=consts_p[:, :])
            W1 = CONSTS[:, 0:P]
            W2 = CONSTS[:, P : 2 * P]
            IDENT = CONSTS[:, 2 * P : 3 * P]

            VS = [None] * n_g
            SS = [None] * n_g

            def issue_load(tg):
                vtile = vpool.tile([P, 16, ST], BF16, tag="v")
                VS[tg] = vtile
                if tg == 0:
                    # split the first load into gi-pair chunks in consumption
                    # order so A-unit(0,0) starts after ~250KB, not 2MB
                    for gp in range(8):
                        nc.sync.dma_start(
                            out=vtile[:, gp * 2 : gp * 2 + 2, :],
                            in_=vt_p[:, tg, gp * 2 : gp * 2 + 2, :],
                        )
                else:
                    nc.sync.dma_start(out=vtile, in_=vt_p[:, tg, :, :])

            def emit_a_unit(tg, gp):
                V = VS[tg]
                S = SS[tg]
                A = apsum.tile([P, 2, ST], F32, tag="a")
                for k in range(2):
                    gi = gp * 2 + k
                    nc.tensor.matmul(
                        A[:, k], W1, V[:, gi, :], start=True, stop=True
                    )
                # 1024-elem f32->bf16 drain; ACT-biased split (DVE is
                # reserved mostly for the 2x-rate bf16 tp2 drains)
                dst = S[:, :, gp * 2 : gp * 2 + 2, :]
                src = A.rearrange("p k (c t) -> p c k t", t=8)
                if gp % 4 == 0:
                    nc.vector.tensor_copy(out=dst, in_=src)
                else:
                    nc.scalar.activation(
                        out=dst,
                        in_=src,
                        func=mybir.ActivationFunctionType.Copy,
                    )
                if gp == 7:
                    VS[tg] = None

            OB = [None, None]  # OUT, vt2 of the group in stage B

            def emit_b_block(tg, B):
                S = SS[tg]
                # chunk c = tokens [8c, 8c+8); 8 blocks of 8 chunks
                if B == 0:
                    OUT = opool.tile([P, 64, P], BF16, tag="o")
                    vt2 = v2pool.tile([P, 64, P], BF16, tag="v2")
                    OB[0], OB[1] = OUT, vt2
                OUT, vt2 = OB
                tp2 = tpsum.tile([P, 8, P], BF16, tag="t2")
                for cs in range(8):
                    c = B * 8 + cs
                    nc.tensor.transpose(tp2[:, cs], S[:, c, :, :], IDENT)
                # 1024-elem bf16 drain: always DVE (2x rate on bf16 PSUM)
                nc.vector.tensor_copy(
                    out=vt2[:, B * 8 : B * 8 + 8, :], in_=tp2
                )
                for k in range(2):
                    Q = B * 2 + k
                    m2 = mpsum.tile([P, 4, P], F32, tag="m2")
                    nc.tensor.matmul(
                        m2,
                        W2,
                        vt2[:, 4 * Q : 4 * Q + 4, :],
                        start=True,
                        stop=True,
                    )
                    # 512-elem f32 drain, alternating engines
                    if Q % 2 == 0:
                        nc.scalar.activation(
                            out=OUT[:, Q * 4 : Q * 4 + 4, :],
                            in_=m2,
                            func=mybir.ActivationFunctionType.Copy,
                        )
                    else:
                        nc.vector.tensor_copy(
                            out=OUT[:, Q * 4 : Q * 4 + 4, :], in_=m2
                        )
                last = tg == n_g - 1
                if last:
                    # taper the tail: store per block on the final group
                    nc.sync.dma_start(
                        out=out_p[:, tg, B * 8 : B * 8 + 8, :],
                        in_=OUT[:, B * 8 : B * 8 + 8, :],
                    )
                elif B % 2 == 1:
                    nc.sync.dma_start(
                        out=out_p[:, tg, (B - 1) * 8 : B * 8 + 8, :],
                        in_=OUT[:, (B - 1) * 8 : B * 8 + 8, :],
                    )
                if B == 7:
                    SS[tg] = None

            # fine-grained software pipeline: A-unit(tg, i) interleaved with
            # B-block(tg-1, i) so DVE/ACT see an even drain mix throughout
            issue_load(0)
            for tg in range(n_g):
                if tg + 1 < n_g:
                    issue_load(tg + 1)
                S = spool.tile([P, 64, 16, 8], BF16, tag="s")
                SS[tg] = S
                for i in range(8):
                    if tg > 0:
                        emit_b_block(tg - 1, i)
                    emit_a_unit(tg, i)
            for i in range(8):
                emit_b_block(n_g - 1, i)

    import bass_rust

    bass_rust.move_matmul_waits_to_ldweights(nc.m)
    bass_rust.generate_event_semaphores(nc)
    return nc


_CACHE = {}


def _make_in_maps(inputs) -> list:
    value = np.asarray(inputs["value"])
    assert value.shape == (T_FULL, N), value.shape
    vb = value.astype(ml_dtypes.bfloat16)
    consts = _host_consts()
    in_maps = []
    for c in range(N_CORES):
        vc = vb[c * T_CORE : (c + 1) * T_CORE]
        # [p, tg, gi, tt] = V[tg*512+tt, gi*128+p]
        vt4 = np.ascontiguousarray(
            vc.reshape(N_G, ST, 16, P).transpose(3, 0, 2, 1)
        )
        in_maps.append({"vt": vt4, "consts": consts})
    return in_maps


def _decode_out(res_list) -> np.ndarray:
    """[w=(j,t8), tg, c, q] per core -> full (T_FULL, N) f32."""
    outs = []
    for r in res_list:
        o = np.asarray(r["out"])  # [128, 8, 64, 128] bf16
        o5 = o.reshape(16, 8, N_G, 64, P)  # [j, t8, tg, c, q]
        o5 = o5.transpose(2, 3, 1, 0, 4)  # [tg, c, t8, j, q]
        outs.append(
            np.ascontiguousarray(o5, dtype=np.float32).reshape(T_CORE, N)
        )
    return np.concatenate(outs, axis=0)


def _probe_ok(out: np.ndarray, inputs, n_rows: int = 3) -> bool:
    value = np.asarray(inputs["value"])
    weight = np.asarray(inputs["weight"], dtype=np.float32)
    rows = np.linspace(0, T_FULL - 1, n_rows).astype(int)
    scale = np.float32(1.0 / np.sqrt(np.float32(weight.shape[0])))
    want = (np.asarray(value[rows], dtype=np.float32) @ weight) * scale
    got = out[rows]
    denom = max(float(np.abs(want).max()), 1e-30)
    rel = float(np.abs(got - want).max()) / denom
    return rel < 1.5e-2


def kernel(**inputs) -> np.ndarray:
    if "nc" not in _CACHE:
        _CACHE["nc"] = build_bass(T_CORE)
    nc = _CACHE["nc"]

    in_maps = _make_in_maps(inputs)
    try:
        out = None
        for attempt in range(2):
            res = run_bass_kernel_spmd(nc, in_maps, list(range(N_CORES)))
            o = _decode_out(res.results)
            if _probe_ok(o, inputs):
                out = o
                break
            print("kernel: probe mismatch on attempt", attempt, flush=True)
        if out is None:
            raise RuntimeError("bass kernel failed host probe twice")
        return out
    except Exception:
        import traceback

        traceback.print_exc()
        print("kernel: falling back to jax path", flush=True)
        import jax
        import jax.numpy as jnp

        value = np.asarray(inputs["value"], dtype=np.float32)
        devs = jax.devices()[:N_CORES]
        scale = np.float32(1.0 / np.sqrt(np.float32(N)))
        w = np.asarray(inputs["weight"], dtype=np.float32)
        outs = []
        for c in range(N_CORES):
            d = devs[c % len(devs)]
            f = jax.jit(lambda a, b: jnp.dot(a, b) * scale, device=d)
            outs.append(f(value[c * T_CORE : (c + 1) * T_CORE], w))
        return np.concatenate([np.asarray(o) for o in outs], axis=0).astype(
            np.float32
        )
